# revision 1
# baseline (speedup 1.0000x reference)
"""Trainium2 Bass kernel for nn_Patchifier (grid-cell argmax + clamped top-k).

Computation per (b, n) map of shape [480, 640] (128 maps total):
  - split into 10x10 grid of 48x64 cells; per-cell argmax (first occurrence,
    row-major within the cell)
  - clamp argmax coords to [1, 478] x [1, 638]; re-read score at the clamped
    coordinate
  - top-80 of the 100 candidate scores per map, sorted descending with
    jax.lax.top_k tie order (lower candidate index first)
Returns (x_coords int32 [4,32,80], y_coords int32, top_scores f32).

Distribution: 128 maps sharded 16-per-core across 8 NeuronCores (pure data
parallel, no collectives).

Per-core dataflow (maps streamed in groups):
  1. HBM -> SBUF raw tile [120, 2560] (partition = 4 consecutive rows)
  2. DVE reduce_max over 64-col chunks -> per-row/per-grid-col maxes
     [120, (4r, 10gc)]
  3. PE-transpose the rowmax block to [40=(gc,r), 120=(g,s)], then one
     contiguous-ish DMA to a DRAM relay laid out as [cell, 12r+s], and read
     back as [100 cells, 48] (partition = cell in reference row-major order)
  4. cell max via reduce; first row attaining it via a value-iota min trick
     (value 4s+r, exact first-occurrence tie semantics)
  5. indirect-DMA gather of the winning row's 64-col chunk (and of the
     row-clamped row) straight from the input in HBM
  6. max + max_index on the gathered chunk -> c*; clamp; score extracted from
     the clamped chunk via a one-hot compare + multiply-accumulate
  7. PE-transpose candidates to [map, 100], 10 rounds of
     max/max_index/match_replace -> sorted top-80 (exact jax tie order)
  8. winner (x, y) coordinates selected on the TensorE via a one-hot
     (iota == broadcast top-index) matmul per map
"""

import sys

if "/opt/trn_rl_repo" not in sys.path:
    sys.path.insert(0, "/opt/trn_rl_repo")

import numpy as np

import concourse.bacc as bacc
import concourse.bass as bass
import concourse.mybir as mybir
from concourse.bass import IndirectOffsetOnAxis
from concourse.tile import TileContext
from concourse.bass_utils import run_bass_kernel_spmd

F32 = mybir.dt.float32
I32 = mybir.dt.int32
U32 = mybir.dt.uint32

N_CORES = 8
MAPS = 16          # maps per core
GROUP = 1          # maps per streaming group
H, W = 480, 640
G = 10             # grid
CH, CW = 48, 64    # cell size
NCAND = 100        # candidates (cells) per map
TOPN = 80
AX = mybir.AluOpType


def _consts():
    """Constant input tensors (identical on every core).

    Candidate partition order is reference row-major: p = g*10 + gc.
    """
    p = np.arange(NCAND)
    g = p // G
    gc = p % G
    c = {}
    c["cgv48"] = (g * CH).astype(np.float32).reshape(NCAND, 1)
    c["cgcv"] = gc.astype(np.float32).reshape(NCAND, 1)
    c["cgc64"] = (gc * CW).astype(np.float32).reshape(NCAND, 1)
    c["mapoff"] = np.broadcast_to(
        (np.arange(MAPS) * (H * W // CW)).astype(np.float32), (NCAND, MAPS)
    ).copy()
    c["iota64"] = np.broadcast_to(
        np.arange(CW, dtype=np.float32), (NCAND, CW)
    ).copy()
    # relay free index f = 12*r + s  ->  row 4*s + r, biased by -1000 so the
    # masked min-trick ignores non-attaining zeros
    f = np.arange(CH)
    c["viota"] = np.broadcast_to(
        (4 * (f % 12) + f // 12 - 1000).astype(np.float32), (NCAND, CH)
    ).copy()
    c["iota100"] = p.astype(np.float32).reshape(NCAND, 1)
    c["ones1"] = np.ones((1, 128), dtype=np.float32)
    c["ident"] = np.eye(128, dtype=np.float32)
    return c


def build_nc():
    nc = bacc.Bacc()

    score = nc.dram_tensor("score", [MAPS, H, W], F32, kind="ExternalInput")
    cdecl = {}
    for name, shape in [
        ("cgv48", [NCAND, 1]), ("cgcv", [NCAND, 1]), ("cgc64", [NCAND, 1]),
        ("mapoff", [NCAND, MAPS]), ("iota64", [NCAND, CW]),
        ("viota", [NCAND, CH]), ("iota100", [NCAND, 1]),
        ("ones1", [1, 128]), ("ident", [128, 128]),
    ]:
        cdecl[name] = nc.dram_tensor(name, shape, F32, kind="ExternalInput")

    x_out = nc.dram_tensor("x_out", [MAPS, TOPN], I32, kind="ExternalOutput")
    y_out = nc.dram_tensor("y_out", [MAPS, TOPN], I32, kind="ExternalOutput")
    s_out = nc.dram_tensor("s_out", [MAPS, TOPN], F32, kind="ExternalOutput")

    # internal DRAM scratch
    relay = nc.dram_tensor("relay", [MAPS, NCAND * CH], F32)

    # flat chunk view of the input: chunk i = 64 contiguous floats
    score_chunks = score[:, :, :].rearrange("m r (gc c) -> (m r gc) c", c=CW)

    with TileContext(nc) as tc:
        with (
            tc.tile_pool(name="raw", bufs=4) as rawp,
            tc.tile_pool(name="rm", bufs=4) as rmp,
            tc.tile_pool(name="small", bufs=1) as sp,
            tc.tile_pool(name="psum", bufs=2, space="PSUM") as pp,
            tc.tile_pool(name="psum1", bufs=1, space="PSUM") as pp1,
        ):
            # issue the first map load before anything else
            raw0 = rawp.tile([120, GROUP, 2560], F32, tag="raw")
            nc.sync.dma_start(
                out=raw0[:, :, :],
                in_=score[0:GROUP, :, :].rearrange(
                    "mm (p rr) c -> p mm (rr c)", rr=4
                ),
            )

            # constants
            cst = {}
            for name, d in cdecl.items():
                t = sp.tile(list(d.shape), F32, tag=name)
                nc.sync.dma_start(out=t[:, :], in_=d[:, :])
                cst[name] = t

            # per-candidate state, all maps (free dim = map where applicable)
            cells48 = sp.tile([NCAND, MAPS * CH], F32, tag="cells48")
            cellmaxf = sp.tile([NCAND, MAPS], F32, tag="cellmaxf")
            rminf = sp.tile([NCAND, MAPS], F32, tag="rminf")
            rstarf = sp.tile([NCAND, MAPS], F32, tag="rstarf")
            a8 = sp.tile([NCAND, MAPS * 8], F32, tag="a8")
            c8 = sp.tile([NCAND, MAPS * 8], U32, tag="c8")
            cstarf = sp.tile([NCAND, MAPS], F32, tag="cstarf")
            rowA = sp.tile([NCAND, MAPS], F32, tag="rowA")
            tmp1 = sp.tile([NCAND, MAPS], F32, tag="tmp1")
            chAi = sp.tile([NCAND, MAPS], I32, tag="chAi")
            chBi = sp.tile([NCAND, MAPS], I32, tag="chBi")
            colA = sp.tile([NCAND, MAPS], F32, tag="colA")
            ccl = sp.tile([NCAND, MAPS], F32, tag="ccl")
            chunkA = sp.tile([NCAND, MAPS * CW], F32, tag="chunkA")
            chunkB = sp.tile([NCAND, MAPS * CW], F32, tag="chunkB")
            junkt = [
                sp.tile([NCAND, CW], F32, tag=f"junk{i}", name=f"junk{i}")
                for i in range(4)
            ]
            tmp1t = [
                sp.tile([NCAND, MAPS], F32, tag=f"tmp1x{i}", name=f"tmp1x{i}")
                for i in range(2)
            ]
            # S3: [scores | x | y] stacked along free dim
            S3 = sp.tile([NCAND, 3 * MAPS], F32, tag="S3")

            ident = cst["ident"]

            n_groups = MAPS // GROUP
            for grp in range(n_groups):
                m0 = grp * GROUP
                if grp == 0:
                    raw = raw0
                else:
                    raw = rawp.tile([120, GROUP, 2560], F32, tag="raw")
                    nc.sync.dma_start(
                        out=raw[:, :, :],
                        in_=score[m0:m0 + GROUP, :, :].rearrange(
                            "mm (p rr) c -> p mm (rr c)", rr=4
                        ),
                    )

                # physical layout (gc, r) so the PE transpose sees one
                # contiguous free dim
                rowmax = rmp.tile([120, GROUP, G, 4], F32, tag="rowmax")
                for k in range(GROUP):
                    nc.vector.reduce_max(
                        out=rowmax[:, k].rearrange("p gc r -> p r gc"),
                        in_=raw[:, k, :].rearrange(
                            "p (r gc c) -> p r gc c", r=4, gc=G, c=CW
                        ),
                        axis=mybir.AxisListType.X,
                    )

                for k in range(GROUP):
                    m = m0 + k
                    # [120, (4r, 10gc)] -> PSUM [40=(gc,r), 120=(g,s)]
                    ps_rt = pp.tile([40, 120], F32, tag="ps_rt")
                    nc.tensor.transpose(
                        out=ps_rt[:, :],
                        in_=rowmax[:, k].rearrange("p gc r -> p (gc r)"),
                        identity=ident[:120, :120],
                    )
                    rt = rmp.tile([40, 120], F32, tag="rt")
                    nc.scalar.copy(out=rt[:, :], in_=ps_rt[:, :])
                    # DRAM relay: addr = 480g + 48gc + 12r + s
                    #           = 48*cell + (12r + s), cell = 10g + gc
                    nc.scalar.dma_start(
                        out=relay[m:m + 1, :].rearrange(
                            "mm (g2 q s) -> (mm q) g2 s", g2=G, q=40, s=12
                        ),
                        in_=rt[:, :].rearrange("q (g2 s) -> q g2 s", g2=G),
                    )
                    # read back cell-major: [100 cells, 48]
                    nc.scalar.dma_start(
                        out=cells48[:, m * CH:(m + 1) * CH],
                        in_=relay[m:m + 1, :].rearrange(
                            "mm (cell f) -> (mm cell) f", cell=NCAND
                        ),
                    )

                sl = slice(m0, m0 + GROUP)
                # cell max over the 48 relayed row-maxes
                nc.vector.reduce_max(
                    out=cellmaxf[:, sl],
                    in_=cells48[:, m0 * CH:(m0 + GROUP) * CH].rearrange(
                        "p (m f) -> p m f", f=CH
                    ),
                    axis=mybir.AxisListType.X,
                )
                # first row attaining the max: min over (row-1000) where
                # value == max (viota carries 4s+r-1000 in relay order)
                for k in range(GROUP):
                    m = m0 + k
                    j48 = junkt[m % 4][:, :CH]
                    nc.vector.scalar_tensor_tensor(
                        out=j48, in0=cells48[:, m * CH:(m + 1) * CH],
                        scalar=cellmaxf[:, m:m + 1],
                        in1=cst["viota"][:, :], op0=AX.is_ge, op1=AX.mult)
                    nc.vector.tensor_reduce(
                        out=rminf[:, m:m + 1], in_=j48,
                        axis=mybir.AxisListType.X, op=AX.min)
                nc.vector.tensor_scalar_add(rstarf[:, sl], rminf[:, sl], 1000.0)

                # global argmax row, and its clamp (also the x coordinate)
                nc.vector.tensor_scalar_add(
                    rowA[:, sl], rstarf[:, sl], cst["cgv48"][:, :])
                xsl = S3[:, MAPS + m0:MAPS + m0 + GROUP]
                nc.vector.tensor_scalar_max(xsl, rowA[:, sl], 1.0)
                nc.vector.tensor_scalar_min(xsl, xsl, float(H - 2))
                # chunk index of the argmax row / clamped row
                tmpg = tmp1t[grp % 2]
                nc.vector.scalar_tensor_tensor(
                    out=tmpg[:, sl], in0=rowA[:, sl], scalar=float(G),
                    in1=cst["mapoff"][:, sl], op0=AX.mult, op1=AX.add)
                nc.vector.tensor_scalar_add(
                    tmpg[:, sl], tmpg[:, sl], cst["cgcv"][:, :])
                nc.vector.tensor_copy(chAi[:, sl], tmpg[:, sl])
                nc.vector.scalar_tensor_tensor(
                    out=tmpg[:, sl], in0=xsl, scalar=float(G),
                    in1=cst["mapoff"][:, sl], op0=AX.mult, op1=AX.add)
                nc.vector.tensor_scalar_add(
                    tmpg[:, sl], tmpg[:, sl], cst["cgcv"][:, :])
                nc.vector.tensor_copy(chBi[:, sl], tmpg[:, sl])

                # HW indirect DMA supports exactly one index per partition
                for m in range(m0, m0 + GROUP):
                    nc.gpsimd.indirect_dma_start(
                        out=chunkA[:, m * CW:(m + 1) * CW], out_offset=None,
                        in_=score_chunks,
                        in_offset=IndirectOffsetOnAxis(ap=chAi[:, m:m + 1], axis=0),
                    )
                    nc.gpsimd.indirect_dma_start(
                        out=chunkB[:, m * CW:(m + 1) * CW], out_offset=None,
                        in_=score_chunks,
                        in_offset=IndirectOffsetOnAxis(ap=chBi[:, m:m + 1], axis=0),
                    )

                for m in range(m0, m0 + GROUP):
                    ca = chunkA[:, m * CW:(m + 1) * CW]
                    nc.vector.max(out=a8[:, m * 8:m * 8 + 8], in_=ca)
                    nc.vector.max_index(
                        out=c8[:, m * 8:m * 8 + 8],
                        in_max=a8[:, m * 8:m * 8 + 8],
                        in_values=ca,
                    )
                    nc.vector.tensor_copy(
                        cstarf[:, m:m + 1], c8[:, m * 8:m * 8 + 1]
                    )

                # global argmax col, clamped (y coordinate), and the local
                # column within the cell after clamping
                nc.vector.tensor_scalar_add(
                    colA[:, sl], cstarf[:, sl], cst["cgc64"][:, :])
                ysl = S3[:, 2 * MAPS + m0:2 * MAPS + m0 + GROUP]
                nc.vector.tensor_scalar_max(ysl, colA[:, sl], 1.0)
                nc.vector.tensor_scalar_min(ysl, ysl, float(W - 2))
                nc.vector.tensor_scalar(
                    out=ccl[:, sl], in0=ysl,
                    scalar1=cst["cgc64"][:, :], scalar2=None, op0=AX.subtract)

                # candidate score = chunkB[ccl] via one-hot + mult + accumulate
                for m in range(m0, m0 + GROUP):
                    nc.vector.scalar_tensor_tensor(
                        out=junkt[2 + m % 2][:, :CW], in0=cst["iota64"][:, :],
                        scalar=ccl[:, m:m + 1],
                        in1=chunkB[:, m * CW:(m + 1) * CW],
                        op0=AX.is_equal, op1=AX.mult,
                        accum_out=S3[:, m:m + 1],
                    )

            # ---- tail: transpose candidates to [map, cand] and do top-k ----
            s3t = pp1.tile([3 * MAPS, NCAND], F32, tag="s3t")
            nc.tensor.transpose(
                out=s3t[:, :], in_=S3[:, :], identity=ident[:NCAND, :NCAND]
            )
            T3 = sp.tile([3 * MAPS, NCAND], F32, tag="T3")
            nc.scalar.copy(out=T3[:, :], in_=s3t[:, :])

            work = sp.tile([MAPS, NCAND], F32, tag="work")
            tops = sp.tile([MAPS, TOPN], F32, tag="tops")
            tidx = sp.tile([MAPS, TOPN], U32, tag="tidx")
            nc.vector.tensor_copy(work[:, :], T3[0:MAPS, :])
            for k in range(TOPN // 8):
                o = k * 8
                nc.vector.max(out=tops[:, o:o + 8], in_=work[:, :])
                nc.vector.max_index(
                    out=tidx[:, o:o + 8], in_max=tops[:, o:o + 8],
                    in_values=work[:, :],
                )
                if k < TOPN // 8 - 1:
                    nc.vector.match_replace(
                        out=work[:, :], in_to_replace=tops[:, o:o + 8],
                        in_values=work[:, :], imm_value=-1e30,
                    )

            # winners' (x, y) via one-hot matmul gather on the PE
            tidxf = sp.tile([MAPS, TOPN], F32, tag="tidxf")
            nc.vector.tensor_copy(tidxf[:, :], tidx[:, :])
            # flatten winner indices onto partition 0 (matmul operands must
            # start at partition 0)
            tidxs = sp.tile([1, MAPS * TOPN], F32, tag="tidxs")
            nc.scalar.dma_start(
                out=tidxs[0:1, :].rearrange("o (m j) -> o m j", m=MAPS),
                in_=tidxf[:, :],
            )
            xyg_ps = pp1.tile([TOPN, 2 * MAPS], F32, tag="xyg_ps")
            s3v = S3[:, :].rearrange("p (f m) -> p m f", f=3)
            for m in range(MAPS):
                bc = pp.tile([NCAND, TOPN], F32, tag="bc")
                nc.tensor.matmul(
                    out=bc[:, :], lhsT=cst["ones1"][0:1, :NCAND],
                    rhs=tidxs[0:1, m * TOPN:(m + 1) * TOPN],
                    start=True, stop=True)
                oh = sp.tile([NCAND, TOPN], F32, tag=f"oh{m % 2}")
                nc.vector.tensor_scalar(
                    out=oh[:, :], in0=bc[:, :], scalar1=cst["iota100"][:, :],
                    scalar2=None, op0=AX.is_equal)
                nc.tensor.matmul(
                    out=xyg_ps[:, 2 * m:2 * m + 2], lhsT=oh[:, :],
                    rhs=s3v[:, m:m + 1, 1:3].squeeze(1),
                    start=True, stop=True)
            xyg = sp.tile([TOPN, 2 * MAPS], F32, tag="xyg")
            nc.scalar.copy(out=xyg[:, :], in_=xyg_ps[:, :])
            # (m, e) -> (e, m) so the transpose lands x rows 0..15, y 16..31
            xyg2 = sp.tile([TOPN, 2 * MAPS], F32, tag="xyg2")
            nc.scalar.copy(
                out=xyg2[:, :].rearrange("p (e m) -> p e m", e=2),
                in_=xyg[:, :].rearrange("p (m e) -> p m e", e=2).transpose([0, 2, 1]),
            )
            xy_ps = pp1.tile([2 * MAPS, TOPN], F32, tag="xy_ps")
            nc.tensor.transpose(
                out=xy_ps[:, :], in_=xyg2[:, :], identity=ident[:TOPN, :TOPN])
            xyi = sp.tile([2 * MAPS, TOPN], I32, tag="xyi")
            nc.vector.tensor_copy(xyi[:, :], xy_ps[:, :])

            nc.scalar.dma_start(out=x_out[:, :], in_=xyi[0:MAPS, :])
            nc.scalar.dma_start(out=y_out[:, :], in_=xyi[MAPS:2 * MAPS, :])
            nc.scalar.dma_start(out=s_out[:, :], in_=tops[:, :])

    nc.compile()
    return nc


_NC = None


def _get_nc():
    global _NC
    if _NC is None:
        _NC = build_nc()
    return _NC


def kernel(score_maps: np.ndarray, top_n=80, _trace=False):
    score_maps = np.ascontiguousarray(np.asarray(score_maps), dtype=np.float32)
    assert score_maps.shape == (4, 32, H, W), score_maps.shape
    assert int(top_n) == TOPN

    nc = _get_nc()
    consts = _consts()
    flat = score_maps.reshape(4 * 32, H, W)
    in_maps = []
    for c in range(N_CORES):
        m = {"score": np.ascontiguousarray(flat[c * MAPS:(c + 1) * MAPS])}
        m.update(consts)
        in_maps.append(m)

    res = run_bass_kernel_spmd(nc, in_maps, list(range(N_CORES)), trace=_trace)
    x = np.concatenate([res.results[c]["x_out"] for c in range(N_CORES)])
    y = np.concatenate([res.results[c]["y_out"] for c in range(N_CORES)])
    s = np.concatenate([res.results[c]["s_out"] for c in range(N_CORES)])
    return (
        x.reshape(4, 32, TOPN).astype(np.int32),
        y.reshape(4, 32, TOPN).astype(np.int32),
        s.reshape(4, 32, TOPN).astype(np.float32),
    )

